# revision 40
# baseline (speedup 1.0000x reference)
"""CompGCN classifier TRN2 kernel — bf16, one-hot PSUM pipeline, 2 launches.

Math: msg = gelu(nfW1[src] + nfW2[tgt] + Rrel[lab]); agg = seg_sum(msg, tgt);
nfu = GRUCell(agg, nf); out = gelu(U1[src]+U2[tgt]+ef@W3^T+b1) @ W2^T + b2
with U1/U2 = nfu @ cls_W1 column-blocks (table precompute folds all per-edge
384->128 matmuls into node-level tables).

Gather strategy: the SWDGE indirect-DMA ucode costs ~1.4us per call
(~1us fixed + ~0.3us pitch, nearly independent of descriptor count) and
consumes ONE index per partition, then streams L consecutive table rows
into L adjacent 128-col tile columns (hardware-measured semantics; the
interp's 128xT multi-index reading does NOT match HW).  So the per-core
gather table (nfW1 rows for launch A, U1 rows for launch B) is laid out
in a custom per-core order with up to CAP duplicate rows per node such
that, per 128-node tgt-block, 1024 edges form 128 runs of RUNL=8
consecutive table rows -> ONE indirect call covers 8 tile columns.
Remaining edges (srcs over the dup cap / partial runs) use classic 1-col
128-index calls.  ~3 calls/block instead of ~10 -> the per-launch gather
stream drops from ~700us to ~290us (A) / ~250us (B); launch A is then
tensor-engine-bound in the edge loop, launch B gather-bound.
Baseline 1604us -> ~950us.

Host prep (index/layout only): relabel nodes (degree-balanced bin packing
across 8 cores x 49 blocks), sort edges by tgt, mark each edge's src
occurrence (< CAP -> owns a table-row copy), pack runs + singles, build
the per-core row list (node ids, with duplicates), slot metadata
(replicated one-hot comparand rows trelB/labB), and permute nf / U1 rows
into table order.  All weights/features cast to bf16.
"""
import sys

sys.path.insert(0, '/opt/trn_rl_repo')
sys.path.insert(0, '/root/.axon_site')

import numpy as np
import ml_dtypes
import concourse.bass as bass
import concourse.mybir as mybir
import concourse.tile as tile
import concourse.bass_utils as bu
from concourse.masks import make_identity

bu.upload_artifacts = lambda tmpdir: 'local://' + tmpdir

bf = ml_dtypes.bfloat16

N, E, D = 50000, 500000, 128
NREL, NCLS = 64, 16
NC = 8
NPC = N // NC            # 6250
NB = 49                  # 128-node blocks per core
NODES_PAD = NB * 128     # 6272
NPAD_G = 50176           # 392 * 128
MAXTPB = 14
BLK_CAP = MAXTPB * 128
RUNL = 8                 # rows streamed per index in a run-call
CAP = 3                  # max table-row copies per node per core
GBLK = 7                 # blocks per run-table group
GRP = NB // GBLK         # 7 run-table groups

dt = mybir.dt
F32 = dt.float32
I32 = dt.int32
BF16 = dt.bfloat16

TRACE = False
LAST_EXEC_NS = {}


def _split_multiwaits(nc, max_waits=1):
    for b in nc.m.functions[0].blocks:
        old = list(b.instructions)
        new = []
        changed = False
        for inst in old:
            si = inst.sync_info
            if si is not None and len(si.on_wait) > max_waits:
                waits = list(si.on_wait)
                chunks = [waits[i:i + max_waits]
                          for i in range(0, len(waits), max_waits)]
                for k, ch in enumerate(chunks[:-1]):
                    new.append(mybir.InstNoOp(
                        name=f"{inst.name}_sw{k}", engine=inst.engine,
                        bass_nofuse=True,
                        sync_info=mybir.SyncInfo(on_wait=ch, on_update=[])))
                inst.sync_info = mybir.SyncInfo(
                    on_wait=chunks[-1], on_update=list(si.on_update))
                changed = True
            new.append(inst)
        if changed:
            b.instructions = new


def _make_nc():
    return bass.Bass("TRN2", target_bir_lowering=False, debug=False,
                     num_devices=NC)


def _build_A(CALLS, CB, TS, TILES, TROWS):
    """CALLS[b]: list of (c0, L, ixcol); CB[b]: cols; TS[b]: col start."""
    nc = _make_nc()
    NIX = max(ix for calls in CALLS for (_, _, ix) in calls) + 1
    nfT = nc.dram_tensor("nfT", (D, TROWS), BF16, kind="ExternalInput")
    nfTl = nc.dram_tensor("nfTl", (D, NODES_PAD), BF16, kind="ExternalInput")
    W1mT = nc.dram_tensor("W1mT", (D, D), BF16, kind="ExternalInput")
    W2mT = nc.dram_tensor("W2mT", (D, D), BF16, kind="ExternalInput")
    Rrel = nc.dram_tensor("Rrel", (NREL, D), BF16, kind="ExternalInput")
    wihT = nc.dram_tensor("wihT", (D, 3 * D), BF16, kind="ExternalInput")
    whhT = nc.dram_tensor("whhT", (D, 3 * D), BF16, kind="ExternalInput")
    gbr = nc.dram_tensor("gbr", (D, 1), F32, kind="ExternalInput")
    gbz = nc.dram_tensor("gbz", (D, 1), F32, kind="ExternalInput")
    gbin = nc.dram_tensor("gbin", (D, 1), F32, kind="ExternalInput")
    gbhn = nc.dram_tensor("gbhn", (D, 1), F32, kind="ExternalInput")
    W1cT = nc.dram_tensor("W1cT", (D, D), BF16, kind="ExternalInput")
    W2cT = nc.dram_tensor("W2cT", (D, D), BF16, kind="ExternalInput")
    srcix = nc.dram_tensor("srcix", (D, NIX), I32, kind="ExternalInput")
    trl = nc.dram_tensor("trl", (D, TILES), F32, kind="ExternalInput")
    trelB = nc.dram_tensor("trelB", (D, TILES * D), BF16, kind="ExternalInput")
    labB = nc.dram_tensor("labB", (NREL, TILES * D), BF16, kind="ExternalInput")
    U1s = nc.dram_tensor("U1s", (NB, D, D), BF16, kind="ExternalOutput")
    U2s = nc.dram_tensor("U2s", (NB, D, D), BF16, kind="ExternalOutput")
    nfW1d = nc.dram_tensor("nfW1d", (TROWS, D), BF16, kind="Internal")

    with tile.TileContext(nc) as tc:
        with tc.tile_pool(name="const", bufs=1) as cp:
            w1m = cp.tile([D, D], BF16); nc.sync.dma_start(w1m[:], W1mT[:])
            w2m = cp.tile([D, D], BF16); nc.sync.dma_start(w2m[:], W2mT[:])
            wih = cp.tile([D, 3 * D], BF16); nc.sync.dma_start(wih[:], wihT[:])
            whh = cp.tile([D, 3 * D], BF16); nc.sync.dma_start(whh[:], whhT[:])
            gbrt = cp.tile([D, 1], F32); nc.sync.dma_start(gbrt[:], gbr[:])
            gbzt = cp.tile([D, 1], F32); nc.sync.dma_start(gbzt[:], gbz[:])
            gin = cp.tile([D, 1], F32); nc.sync.dma_start(gin[:], gbin[:])
            ghn = cp.tile([D, 1], F32); nc.sync.dma_start(ghn[:], gbhn[:])
            w1c = cp.tile([D, D], BF16); nc.sync.dma_start(w1c[:], W1cT[:])
            w2c = cp.tile([D, D], BF16); nc.sync.dma_start(w2c[:], W2cT[:])
            rrl = cp.tile([NREL, D], BF16); nc.sync.dma_start(rrl[:], Rrel[:])
            six = cp.tile([D, NIX], I32); nc.sync.dma_start(six[:], srcix[:])
            trlt = cp.tile([D, TILES], F32); nc.sync.dma_start(trlt[:], trl[:])
            nfl = cp.tile([D, NODES_PAD], BF16)
            nc.sync.dma_start(nfl[:], nfTl[:])
            iotac = cp.tile([D, 1], F32)
            nc.gpsimd.iota(iotac[:], pattern=[[0, 1]], base=0,
                           channel_multiplier=1,
                           allow_small_or_imprecise_dtypes=True)
            iotR = cp.tile([D, D], F32)   # row-iota: iotR[p, c] = c
            nc.gpsimd.iota(iotR[:], pattern=[[1, D]], base=0,
                           channel_multiplier=0,
                           allow_small_or_imprecise_dtypes=True)
            identE = cp.tile([D, D], BF16)
            make_identity(nc, identE[:])
            w2loc = cp.tile([D, NB * D], BF16)   # local nfW2 blocks [n, d]

            # ---- phase A1: per-core nfW1 gather table ----
            with tc.tile_pool(name="pa", bufs=3) as pa, \
                 tc.tile_pool(name="ps_a", bufs=4, space="PSUM") as ps_a:
                for i in range((TROWS + 2047) // 2048):
                    c0 = i * 2048
                    cw = min(2048, TROWS - c0)
                    ch = pa.tile([D, 2048], BF16)
                    nc.sync.dma_start(ch[:, 0:cw], nfT[:, c0:c0 + cw])
                    st = pa.tile([D, 2048], BF16)
                    for h in range(cw // 512):
                        p1 = ps_a.tile([D, 512], F32, space="PSUM")
                        for j in range(4):
                            nc.tensor.matmul(
                                out=p1[:, j * D:(j + 1) * D],
                                lhsT=ch[:, (h * 4 + j) * D:
                                        (h * 4 + j + 1) * D],
                                rhs=w1m[:], start=True, stop=True)
                        if h % 2 == 0:
                            nc.scalar.copy(st[:, h * 512:(h + 1) * 512],
                                           p1[:])
                        else:
                            nc.vector.tensor_copy(
                                st[:, h * 512:(h + 1) * 512], p1[:])
                    nc.sync.dma_start(
                        nfW1d[c0:c0 + cw, :].rearrange(
                            "(b p) d -> p b d", p=D), st[:, 0:cw])
                # ---- phase A2: local nfW2 blocks into SBUF ----
                for i in range(NB // 4 + 1):
                    nblk = min(4, NB - i * 4)
                    if nblk <= 0:
                        break
                    p1 = ps_a.tile([D, 512], F32, space="PSUM")
                    for j in range(nblk):
                        b = i * 4 + j
                        nc.tensor.matmul(
                            out=p1[:, j * D:(j + 1) * D],
                            lhsT=nfl[:, b * D:(b + 1) * D],
                            rhs=w2m[:], start=True, stop=True)
                    nc.vector.tensor_copy(
                        w2loc[:, i * 512:i * 512 + nblk * D],
                        p1[:, 0:nblk * D])

            # ---- phase B: edges + GRU + U tables ----
            with tc.tile_pool(name="pg", bufs=6) as pg, \
                 tc.tile_pool(name="prow", bufs=6) as prow, \
                 tc.tile_pool(name="poh", bufs=4) as poh, \
                 tc.tile_pool(name="pmsg", bufs=3) as pmsg, \
                 tc.tile_pool(name="pgr", bufs=2) as pgr, \
                 tc.tile_pool(name="pub", bufs=2) as pub, \
                 tc.tile_pool(name="ps_pre", bufs=4, space="PSUM") as ps_pre, \
                 tc.tile_pool(name="ps_agg", bufs=2, space="PSUM") as ps_agg, \
                 tc.tile_pool(name="ps_g4", bufs=1, space="PSUM") as ps_g4, \
                 tc.tile_pool(name="ps_u", bufs=1, space="PSUM") as ps_u:
                for b in range(NB):
                    tpb = CB[b]
                    ts = TS[b]
                    nsup = (tpb + 3) // 4
                    g1 = pg.tile([D, tpb * D], BF16)
                    for (c0, L, ixcol) in CALLS[b]:
                        nc.gpsimd.indirect_dma_start(
                            out=g1[:, c0 * D:(c0 + L) * D], out_offset=None,
                            in_=nfW1d[:],
                            in_offset=bass.IndirectOffsetOnAxis(
                                ap=six[:, ixcol:ixcol + 1], axis=0))
                    trB = prow.tile([D, tpb * D], BF16)
                    nc.sync.dma_start(
                        trB[:], trelB[:, ts * D:(ts + tpb) * D])
                    lbB = prow.tile([NREL, tpb * D], BF16)
                    nc.sync.dma_start(
                        lbB[:], labB[:, ts * D:(ts + tpb) * D])
                    agg = ps_agg.tile([D, D], F32, space="PSUM")
                    for s in range(nsup):
                        t0 = s * 4
                        nt = min(4, tpb - t0)
                        w = nt * D
                        ohs = []
                        for (bsrc, npart) in ((trB, D), (lbB, NREL)):
                            oht_ = poh.tile([D, 512], BF16)
                            nc.vector.tensor_scalar(
                                out=oht_[0:npart, 0:w],
                                in0=bsrc[0:npart, t0 * D:t0 * D + w],
                                scalar1=iotac[0:npart, 0:1], scalar2=None,
                                op0=mybir.AluOpType.is_equal)
                            ohs.append(oht_)
                        ohT, ohrT = ohs
                        oh4 = poh.tile([D, 512], BF16)
                        nc.vector.tensor_tensor(
                            out=oh4[:, 0:w].rearrange("p (t d) -> p t d", t=nt),
                            in0=trlt[:, ts + t0:ts + t0 + nt]
                                .unsqueeze(2).broadcast_to((D, nt, D)),
                            in1=iotR[:].unsqueeze(1).broadcast_to((D, nt, D)),
                            op=mybir.AluOpType.is_equal)
                        pre = ps_pre.tile([D, 512], F32, space="PSUM")
                        for j in range(nt):
                            nc.tensor.matmul(
                                out=pre[:, j * D:(j + 1) * D],
                                lhsT=ohT[:, j * D:(j + 1) * D],
                                rhs=w2loc[:, b * D:(b + 1) * D],
                                start=True, stop=False)
                            nc.tensor.matmul(
                                out=pre[:, j * D:(j + 1) * D],
                                lhsT=ohrT[0:NREL, j * D:(j + 1) * D],
                                rhs=rrl[:], start=False, stop=True)
                        s2 = pmsg.tile([D, 512], BF16)
                        nc.vector.tensor_tensor(
                            out=s2[:, 0:w], in0=pre[:, 0:w],
                            in1=g1[:, t0 * D:t0 * D + w],
                            op=mybir.AluOpType.add)
                        msg4 = pmsg.tile([D, 512], BF16)
                        nc.scalar.activation(
                            msg4[:, 0:w], s2[:, 0:w],
                            mybir.ActivationFunctionType.Gelu)
                        for j in range(nt):
                            t = t0 + j
                            nc.tensor.matmul(
                                out=agg[:], lhsT=msg4[:, j * D:(j + 1) * D],
                                rhs=oh4[:, j * D:(j + 1) * D],
                                start=(t == 0), stop=(t == tpb - 1))
                    # ---- GRU (per block, overlaps the gather stream) ----
                    nfb = nfl[:, b * D:(b + 1) * D]
                    aggs = pgr.tile([D, D], BF16)
                    nc.scalar.copy(aggs[:], agg[:])
                    g4 = ps_g4.tile([D, 4 * D], F32, space="PSUM")
                    nc.tensor.matmul(out=g4[:, 0:D], lhsT=wih[:, 0:D],
                                     rhs=aggs[:], start=True, stop=False)
                    nc.tensor.matmul(out=g4[:, 0:D], lhsT=whh[:, 0:D],
                                     rhs=nfb, start=False, stop=True)
                    nc.tensor.matmul(out=g4[:, D:2 * D], lhsT=wih[:, D:2 * D],
                                     rhs=aggs[:], start=True, stop=False)
                    nc.tensor.matmul(out=g4[:, D:2 * D], lhsT=whh[:, D:2 * D],
                                     rhs=nfb, start=False, stop=True)
                    nc.tensor.matmul(out=g4[:, 2 * D:3 * D],
                                     lhsT=wih[:, 2 * D:3 * D],
                                     rhs=aggs[:], start=True, stop=True)
                    nc.tensor.matmul(out=g4[:, 3 * D:4 * D],
                                     lhsT=whh[:, 2 * D:3 * D],
                                     rhs=nfb, start=True, stop=True)
                    rz = pgr.tile([D, 2 * D], BF16)
                    nc.scalar.activation(rz[:, 0:D], g4[:, 0:D],
                                         mybir.ActivationFunctionType.Sigmoid,
                                         bias=gbrt[:, 0:1])
                    nc.scalar.activation(rz[:, D:2 * D], g4[:, D:2 * D],
                                         mybir.ActivationFunctionType.Sigmoid,
                                         bias=gbzt[:, 0:1])
                    r = rz[:, 0:D]
                    z = rz[:, D:2 * D]
                    hn = pgr.tile([D, D], BF16)
                    nc.vector.tensor_scalar(
                        out=hn[:], in0=g4[:, 3 * D:4 * D],
                        scalar1=ghn[:, 0:1], scalar2=None,
                        op0=mybir.AluOpType.add)
                    t1 = pgr.tile([D, D], BF16)
                    nc.vector.tensor_tensor(out=t1[:], in0=r, in1=hn[:],
                                            op=mybir.AluOpType.mult)
                    t2 = pgr.tile([D, D], BF16)
                    nc.vector.tensor_tensor(out=t2[:], in0=t1[:],
                                            in1=g4[:, 2 * D:3 * D],
                                            op=mybir.AluOpType.add)
                    n_ = pgr.tile([D, D], BF16)
                    nc.scalar.activation(n_[:], t2[:],
                                         mybir.ActivationFunctionType.Tanh,
                                         bias=gin[:, 0:1])
                    d1 = pgr.tile([D, D], BF16)
                    nc.vector.tensor_tensor(out=d1[:], in0=nfb, in1=n_[:],
                                            op=mybir.AluOpType.subtract)
                    d2 = pgr.tile([D, D], BF16)
                    nc.vector.tensor_tensor(out=d2[:], in0=z[:], in1=d1[:],
                                            op=mybir.AluOpType.mult)
                    nfu = pgr.tile([D, D], BF16)
                    nc.vector.tensor_tensor(out=nfu[:], in0=n_[:], in1=d2[:],
                                            op=mybir.AluOpType.add)
                    pu = ps_u.tile([D, 2 * D], F32, space="PSUM")
                    nc.tensor.matmul(out=pu[:, 0:D], lhsT=nfu[:], rhs=w1c[:],
                                     start=True, stop=True)
                    nc.tensor.matmul(out=pu[:, D:2 * D], lhsT=nfu[:],
                                     rhs=w2c[:], start=True, stop=True)
                    u1 = pub.tile([D, D], BF16)
                    nc.scalar.copy(u1[:], pu[:, 0:D])
                    nc.sync.dma_start(U1s[b], u1[:])
                    u2 = pub.tile([D, D], BF16)
                    nc.scalar.copy(u2[:], pu[:, D:2 * D])
                    nc.sync.dma_start(U2s[b], u2[:])

    _split_multiwaits(nc)
    return nc


def _build_B(CALLS, CB, TS, TILES, TROWS):
    nc = _make_nc()
    NIX = max(ix for calls in CALLS for (_, _, ix) in calls) + 1
    U1 = nc.dram_tensor("U1", (TROWS, D), BF16, kind="ExternalInput")
    U2l = nc.dram_tensor("U2l", (NB, D, D), BF16, kind="ExternalInput")
    efT = nc.dram_tensor("efT", (D, TILES * D), BF16, kind="ExternalInput")
    srcix = nc.dram_tensor("srcix", (D, NIX), I32, kind="ExternalInput")
    trelB = nc.dram_tensor("trelB", (D, TILES * D), BF16, kind="ExternalInput")
    W3cT = nc.dram_tensor("W3cT", (D, D), BF16, kind="ExternalInput")
    clsW2 = nc.dram_tensor("clsW2", (D, NCLS), BF16, kind="ExternalInput")
    b1 = nc.dram_tensor("b1", (D, 1), F32, kind="ExternalInput")
    b2 = nc.dram_tensor("b2", (NCLS, 1), F32, kind="ExternalInput")
    outT = nc.dram_tensor("outT", (NCLS, TILES * D), F32,
                          kind="ExternalOutput")

    with tile.TileContext(nc) as tc:
        with tc.tile_pool(name="const", bufs=1) as cp, \
             tc.tile_pool(name="pg", bufs=6) as pg, \
             tc.tile_pool(name="prow", bufs=6) as prow, \
             tc.tile_pool(name="pef", bufs=5) as pef, \
             tc.tile_pool(name="poh", bufs=4) as poh, \
             tc.tile_pool(name="phc", bufs=3) as phc, \
             tc.tile_pool(name="pu2", bufs=4) as pu2, \
             tc.tile_pool(name="pout", bufs=3) as pout, \
             tc.tile_pool(name="ps_x", bufs=3, space="PSUM") as ps_x, \
             tc.tile_pool(name="ps_p", bufs=3, space="PSUM") as ps_p:
            w3 = cp.tile([D, D], BF16); nc.sync.dma_start(w3[:], W3cT[:])
            w2 = cp.tile([D, NCLS], BF16); nc.sync.dma_start(w2[:], clsW2[:])
            b1t = cp.tile([D, 1], F32); nc.sync.dma_start(b1t[:], b1[:])
            b2t = cp.tile([NCLS, 1], F32); nc.sync.dma_start(b2t[:], b2[:])
            six = cp.tile([D, NIX], I32); nc.sync.dma_start(six[:], srcix[:])
            iotac = cp.tile([D, 1], F32)
            nc.gpsimd.iota(iotac[:], pattern=[[0, 1]], base=0,
                           channel_multiplier=1,
                           allow_small_or_imprecise_dtypes=True)
            identE = cp.tile([D, D], BF16)
            make_identity(nc, identE[:])

            for b in range(NB):
                tpb = CB[b]
                ts = TS[b]
                nsup = (tpb + 3) // 4
                g1 = pg.tile([D, tpb * D], BF16)
                for (c0, L, ixcol) in CALLS[b]:
                    nc.gpsimd.indirect_dma_start(
                        out=g1[:, c0 * D:(c0 + L) * D], out_offset=None,
                        in_=U1[:],
                        in_offset=bass.IndirectOffsetOnAxis(
                            ap=six[:, ixcol:ixcol + 1], axis=0))
                u2b = pu2.tile([D, D], BF16)
                nc.sync.dma_start(u2b[:], U2l[b])
                ef = pef.tile([D, tpb * D], BF16)
                nc.sync.dma_start(ef[:], efT[:, ts * D:(ts + tpb) * D])
                trB = prow.tile([D, tpb * D], BF16)
                nc.sync.dma_start(trB[:], trelB[:, ts * D:(ts + tpb) * D])
                ob = pout.tile([NCLS, tpb * D], F32)
                for s in range(nsup):
                    t0 = s * 4
                    nt = min(4, tpb - t0)
                    w = nt * D
                    ohT = poh.tile([D, 512], BF16)
                    nc.vector.tensor_scalar(
                        out=ohT[:, 0:w],
                        in0=trB[:, t0 * D:t0 * D + w],
                        scalar1=iotac[:, 0:1], scalar2=None,
                        op0=mybir.AluOpType.is_equal)
                    xt = ps_x.tile([D, 512], F32, space="PSUM")
                    nc.tensor.matmul(
                        out=xt[:, 0:w], lhsT=w3[:],
                        rhs=ef[:, t0 * D:t0 * D + w], start=True, stop=False)
                    nc.tensor.matmul(
                        out=xt[:, 0:w], lhsT=u2b[:],
                        rhs=ohT[:, 0:w], start=False, stop=False)
                    for j in range(nt):
                        t = t0 + j
                        nc.tensor.matmul(
                            out=xt[:, j * D:(j + 1) * D],
                            lhsT=g1[:, t * D:(t + 1) * D],
                            rhs=identE[:], start=False,
                            stop=(j == nt - 1), skip_group_check=True)
                    hc = phc.tile([D, 512], BF16)
                    nc.scalar.activation(
                        hc[:, 0:w], xt[:, 0:w],
                        mybir.ActivationFunctionType.Gelu, bias=b1t[:, 0:1])
                    po = ps_p.tile([NCLS, 512], F32, space="PSUM")
                    nc.tensor.matmul(out=po[:, 0:w], lhsT=w2[:],
                                     rhs=hc[:, 0:w], start=True, stop=True)
                    nc.scalar.add(ob[:, t0 * D:t0 * D + w], po[:, 0:w],
                                  b2t[:, 0:1])
                nc.sync.dma_start(outT[:, ts * D:(ts + tpb) * D], ob[:])

    _split_multiwaits(nc)
    return nc


_CACHE = {}


def _get(name, builder, *args):
    if name not in _CACHE:
        _CACHE[name] = builder(*args)
    return _CACHE[name]


def _run(nc, in_maps, tag):
    kw = {}
    if TRACE:
        import tempfile
        kw = dict(trace=True, tmpdir=tempfile.mkdtemp(prefix=f"gcn2_{tag}_"))
    res = bu.run_bass_kernel_spmd(nc, in_maps, core_ids=list(range(NC)), **kw)
    if TRACE:
        LAST_EXEC_NS[tag] = res.exec_time_ns
        LAST_EXEC_NS[tag + "_dir"] = kw["tmpdir"]
    return res.results


def kernel(node_features, edge_features, edge_index, labels_for_rel_emb,
           rel_emb, msg_W, msg_b, gru_w_ih, gru_w_hh, gru_b_ih, gru_b_hh,
           cls_W1, cls_b1, cls_W2, cls_b2):
    nf = np.asarray(node_features, np.float32)
    ef = np.asarray(edge_features, np.float32)
    src = np.asarray(edge_index[0], np.int64).astype(np.int32)
    tgt = np.asarray(edge_index[1], np.int64).astype(np.int32)
    lab = np.asarray(labels_for_rel_emb, np.int64).astype(np.int32)
    msg_W = np.asarray(msg_W, np.float32)
    cls_W1 = np.asarray(cls_W1, np.float32)

    # ---- host: relabel nodes so per-block edge counts pack tightly ----
    deg = np.bincount(tgt, minlength=N)
    byd = np.argsort(-deg, kind="stable")
    corder = np.concatenate([np.r_[0:NC, NC - 1:-1:-1]
                             for _ in range((N + 2 * NC - 1) // (2 * NC))])
    node_core = np.empty(N, np.int32)
    node_core[byd] = corder[:N]
    perm = np.empty(N, np.int64)  # old id -> new (padded-global) id
    for c in range(NC):
        nodes_c = byd[node_core[byd] == c]
        degs_c = deg[nodes_c]
        cap_e = np.full(NB, float(max(1280, int(np.ceil(degs_c.sum() / NB)))))
        cnt_n = np.zeros(NB, np.int64)
        sum_e = np.zeros(NB, np.float64)
        blk_of = np.empty(len(nodes_c), np.int64)
        for i, dg in enumerate(degs_c):
            cand = np.where((cnt_n < 128) & (sum_e + dg <= cap_e))[0]
            if len(cand) == 0:
                cand = np.where(cnt_n < 128)[0]
                j = cand[np.argmin(sum_e[cand])]
            else:
                j = cand[np.argmax(sum_e[cand])]
            blk_of[i] = j
            cnt_n[j] += 1
            sum_e[j] += dg
        ordb = np.argsort(blk_of, kind="stable")
        pcount = np.zeros(NB, np.int64)
        newloc = np.empty(len(nodes_c), np.int64)
        for i in ordb:
            j = blk_of[i]
            newloc[i] = j * 128 + pcount[j]
            pcount[j] += 1
        perm[nodes_c] = c * NODES_PAD + newloc
    src = perm[src].astype(np.int32)
    tgt = perm[tgt].astype(np.int32)
    nf_pad = np.zeros((NC * NODES_PAD, D), np.float32)
    nf_pad[perm] = nf
    nf = nf_pad

    order = np.argsort(tgt, kind="stable")
    tgt_s = tgt[order]; src_s = src[order]; lab_s = lab[order]
    ef_s = ef[order]
    core_s = tgt_s // NODES_PAD
    blk_s = (tgt_s - core_s * NODES_PAD) // 128
    key = core_s * NB + blk_s
    gstart = np.searchsorted(key, np.arange(NC * NB + 1))
    counts = np.diff(gstart).reshape(NC, NB)

    # src occurrence rank within each core's tgt-sorted edge stream:
    # occ < CAP -> edge owns a fresh table-row copy of its src
    occ = np.zeros(E, np.int64)
    for c in range(NC):
        lo, hi = gstart[c * NB], gstart[(c + 1) * NB] if c + 1 < NC else gstart[NC * NB]
        s_c = src_s[lo:hi]
        so = np.argsort(s_c, kind="stable")
        ss = s_c[so]
        grp = np.concatenate([[0], np.cumsum(ss[1:] != ss[:-1])])
        first = np.concatenate([[0], np.flatnonzero(ss[1:] != ss[:-1]) + 1])
        occ_sorted = np.arange(len(ss)) - first[grp]
        o = np.empty(len(ss), np.int64)
        o[so] = occ_sorted
        occ[lo:hi] = o

    # ---- per-core run/single packing ----
    # per block: pick run length LB[b] (8/4/2) so even thin blocks ride
    # run-calls; NRUN/NSING = max over cores for the shared SPMD structure.
    COB = np.zeros((NC, NB), np.int64)
    for c in range(NC):
        for b in range(NB):
            lo, hi = gstart[c * NB + b], gstart[c * NB + b + 1]
            COB[c, b] = np.count_nonzero(occ[lo:hi] < CAP)
    LB = []
    NRUN = []
    for b in range(NB):
        mx = int(COB[:, b].max())
        for L in (8, 4, 2):
            if mx >= 128 * L:
                break
        LB.append(L)
        NRUN.append(max(1, mx // (128 * L)) if mx >= 128 * L else 0)
    NSING = [0] * NB
    for b in range(NB):
        need = 0
        rs = 128 * LB[b]
        for c in range(NC):
            nedge = int(gstart[c * NB + b + 1] - gstart[c * NB + b])
            placed_full = (min(int(COB[c, b]), NRUN[b] * rs) // LB[b]) * LB[b]
            rest = nedge - placed_full
            need = max(need, (rest + 127) // 128)
        NSING[b] = need

    CB = [NRUN[b] * LB[b] + NSING[b] for b in range(NB)]
    assert max(CB) <= MAXTPB, f"block cols overflow: {max(CB)}"
    TS = np.concatenate([[0], np.cumsum(CB)]).astype(int)
    TILES = int(TS[NB])
    TS = TS[:NB]
    SLOTS = TILES * 128

    # call list (shared across cores): per block: run-calls then single-calls
    CALLS = []
    ixcol = 0
    for b in range(NB):
        calls_b = []
        for r in range(NRUN[b]):
            calls_b.append((r * LB[b], LB[b], ixcol)); ixcol += 1
        for s_ in range(NSING[b]):
            calls_b.append((NRUN[b] * LB[b] + s_, 1, ixcol)); ixcol += 1
        CALLS.append(calls_b)
    NIX = ixcol

    # ---- per-core tables, indices, slot metadata ----
    ROWLISTS = []
    IXCS = np.zeros((NC, D, NIX), np.int32)
    TRELP = np.full((NC, SLOTS), -1.0, np.float32)
    LABF = np.full((NC, SLOTS), -1.0, np.float32)
    EPOS = np.full(E, -1, np.int64)   # edge (tgt-sorted idx) -> core*SLOTS+slot
    for c in range(NC):
        rowlist = []
        rpos = {}                     # node id -> first table row
        for b in range(NB):
            Lb = LB[b]
            rs = 128 * Lb
            lo, hi = gstart[c * NB + b], gstart[c * NB + b + 1]
            eb = np.arange(lo, hi)
            isco = occ[lo:hi] < CAP
            co_e = eb[isco]
            oth_e = eb[~isco]
            placed = min(len(co_e), NRUN[b] * rs)
            run_e = co_e[:placed]
            rest_e = np.concatenate([co_e[placed:], oth_e])
            ci = 0
            # run-calls
            for r in range(NRUN[b]):
                seg = run_e[r * rs:(r + 1) * rs]
                base = len(rowlist)
                for p in range(128):
                    run = seg[p * Lb:(p + 1) * Lb]
                    if len(run) == Lb:
                        IXCS[c, p, CALLS[b][ci][2]] = base + p * Lb
                        for j, e_ in enumerate(run):
                            col = TS[b] + r * Lb + j
                            slot = c * SLOTS + col * 128 + p
                            TRELP[c, col * 128 + p] = tgt_s[e_] - (
                                c * NODES_PAD + b * 128)
                            LABF[c, col * 128 + p] = lab_s[e_]
                            EPOS[e_] = slot
                            s_n = int(src_s[e_])
                            if s_n not in rpos:
                                rpos[s_n] = len(rowlist)
                            rowlist.append(s_n)
                    else:
                        IXCS[c, p, CALLS[b][ci][2]] = 0   # dummy run
                        # partial run edges go to rest
                        rest_e = np.concatenate([rest_e, run])
                        # pad rowlist so later runs keep alignment
                        rowlist.extend([0] * Lb)
                ci += 1
            # single-calls (indices into first-copy rows; append missing)
            for s_ in range(NSING[b]):
                seg = rest_e[s_ * 128:(s_ + 1) * 128]
                col = TS[b] + NRUN[b] * LB[b] + s_
                callix = CALLS[b][ci][2]
                for p in range(128):
                    if p < len(seg):
                        e_ = seg[p]
                        TRELP[c, col * 128 + p] = tgt_s[e_] - (
                            c * NODES_PAD + b * 128)
                        LABF[c, col * 128 + p] = lab_s[e_]
                        EPOS[e_] = c * SLOTS + col * 128 + p
                        s_n = int(src_s[e_])
                        if s_n not in rpos:
                            rpos[s_n] = len(rowlist)
                            rowlist.append(s_n)
                        IXCS[c, p, callix] = rpos[s_n]
                    # else: stays masked, index 0
                ci += 1
        ROWLISTS.append(rowlist)
    assert EPOS.min() >= 0, "some edges were not assigned slots"
    TROWS = max(len(rl) for rl in ROWLISTS)
    TROWS = ((TROWS + 511) // 512) * 512

    nfT16 = np.ascontiguousarray(nf.T).astype(bf)   # [D, NC*NODES_PAD]
    W1mT = np.ascontiguousarray(msg_W[:, 0:D].T).astype(bf)
    W2mT = np.ascontiguousarray(msg_W[:, D:2 * D].T).astype(bf)
    Rrel = (np.asarray(rel_emb, np.float32) @ msg_W[:, 2 * D:3 * D].T
            + np.asarray(msg_b, np.float32)).astype(bf)
    wihT = np.ascontiguousarray(np.asarray(gru_w_ih, np.float32).T).astype(bf)
    whhT = np.ascontiguousarray(np.asarray(gru_w_hh, np.float32).T).astype(bf)
    bih = np.asarray(gru_b_ih, np.float32)
    bhh = np.asarray(gru_b_hh, np.float32)
    gbr = (bih[0:D] + bhh[0:D]).reshape(D, 1).astype(np.float32)
    gbz = (bih[D:2 * D] + bhh[D:2 * D]).reshape(D, 1).astype(np.float32)
    gbin = bih[2 * D:3 * D].reshape(D, 1).astype(np.float32)
    gbhn = bhh[2 * D:3 * D].reshape(D, 1).astype(np.float32)
    W1cT = np.ascontiguousarray(cls_W1[:, 0:D].T).astype(bf)
    W2cT = np.ascontiguousarray(cls_W1[:, D:2 * D].T).astype(bf)
    W3cT = np.ascontiguousarray(cls_W1[:, 2 * D:3 * D].T).astype(bf)

    def col_layout(a):
        return np.ascontiguousarray(a.reshape(TILES, 128).T)

    ROWARR = []
    for c in range(NC):
        ra = np.zeros(TROWS, np.int64)
        rl = ROWLISTS[c]
        ra[:len(rl)] = np.asarray(rl, np.int64)
        ROWARR.append(ra)

    in_maps_A = []
    for c in range(NC):
        nfTl = nfT16[:, c * NODES_PAD:(c + 1) * NODES_PAD]
        m = {
            "nfTl": np.ascontiguousarray(nfTl), "W1mT": W1mT,
            "W2mT": W2mT, "Rrel": Rrel, "wihT": wihT, "whhT": whhT,
            "gbr": gbr, "gbz": gbz, "gbin": gbin, "gbhn": gbhn,
            "W1cT": W1cT, "W2cT": W2cT,
            "srcix": np.ascontiguousarray(IXCS[c]),
            "trl": np.ascontiguousarray(col_layout(TRELP[c])),
            "trelB": np.ascontiguousarray(np.broadcast_to(
                TRELP[c].astype(bf)[None, :], (D, SLOTS))),
            "labB": np.ascontiguousarray(np.broadcast_to(
                LABF[c].astype(bf)[None, :], (NREL, SLOTS))),
        }
        m["nfT"] = np.ascontiguousarray(nfT16[:, ROWARR[c]])
        in_maps_A.append(m)

    ncA = _get("A", _build_A, CALLS, CB, TS, TILES, TROWS)
    resA = _run(ncA, in_maps_A, "A")

    U1 = np.concatenate(
        [np.asarray(resA[c]["U1s"]).reshape(NODES_PAD, D)
         for c in range(NC)], axis=0)    # [NC*NODES_PAD, D], padded-global ids

    clsW2 = np.ascontiguousarray(np.asarray(cls_W2, np.float32).T).astype(bf)
    b1v = np.asarray(cls_b1, np.float32).reshape(D, 1)
    b2v = np.asarray(cls_b2, np.float32).reshape(NCLS, 1)

    EFP = np.zeros((NC * SLOTS, D), np.float32)
    EFP[EPOS] = ef_s

    in_maps_B = []
    for c in range(NC):
        sl = slice(c * SLOTS, (c + 1) * SLOTS)
        u2 = np.asarray(resA[c]["U2s"])
        in_maps_B.append({
            "U1": np.ascontiguousarray(U1[ROWARR[c]]).astype(bf), "U2l": u2,
            "efT": np.ascontiguousarray(EFP[sl].T).astype(bf),
            "srcix": in_maps_A[c]["srcix"],
            "trelB": in_maps_A[c]["trelB"],
            "W3cT": W3cT, "clsW2": clsW2, "b1": b1v, "b2": b2v,
        })

    ncB = _get("B", _build_B, CALLS, CB, TS, TILES, TROWS)
    resB = _run(ncB, in_maps_B, "B")

    outS = np.concatenate(
        [np.asarray(resB[c]["outT"]).T for c in range(NC)], axis=0)
    out_sorted = outS[EPOS]           # [E, NCLS] in tgt-sorted edge order
    out = np.empty((E, NCLS), np.float32)
    out[order] = out_sorted
    return np.ascontiguousarray(out.astype(np.float32))


# revision 43
# speedup vs baseline: 1.1603x; 1.1603x over previous
"""CompGCN classifier TRN2 kernel — bf16, one-hot PSUM pipeline, 2 launches.

Math: msg = gelu(nfW1[src] + nfW2[tgt] + Rrel[lab]); agg = seg_sum(msg, tgt);
nfu = GRUCell(agg, nf); out = gelu(U1[src]+U2[tgt]+ef@W3^T+b1) @ W2^T + b2
with U1/U2 = nfu @ cls_W1 column-blocks (table precompute folds all per-edge
384->128 matmuls into node-level tables).

Gather strategy: the SWDGE indirect-DMA ucode costs ~1.4us per call
(~1us fixed + ~0.3us pitch, nearly independent of descriptor count) and
consumes ONE index per partition, then streams L consecutive table rows
into L adjacent 128-col tile columns (hardware-measured semantics; the
interp's 128xT multi-index reading does NOT match HW).  So the per-core
gather table (nfW1 rows for launch A, U1 rows for launch B) is laid out
in a custom per-core order with up to CAP duplicate rows per node such
that, per 128-node tgt-block, 1024 edges form 128 runs of RUNL=8
consecutive table rows -> ONE indirect call covers 8 tile columns.
Remaining edges (srcs over the dup cap / partial runs) use classic 1-col
128-index calls.  ~3 calls/block instead of ~10 -> the per-launch gather
stream drops from ~700us to ~290us (A) / ~250us (B); launch A is then
tensor-engine-bound in the edge loop, launch B gather-bound.
Baseline 1604us -> ~950us.

Host prep (index/layout only): relabel nodes (degree-balanced bin packing
across 8 cores x 49 blocks), sort edges by tgt, mark each edge's src
occurrence (< CAP -> owns a table-row copy), pack runs + singles, build
the per-core row list (node ids, with duplicates), slot metadata
(replicated one-hot comparand rows trelB/labB), and permute nf / U1 rows
into table order.  All weights/features cast to bf16.
"""
import sys

sys.path.insert(0, '/opt/trn_rl_repo')
sys.path.insert(0, '/root/.axon_site')

import numpy as np
import ml_dtypes
import concourse.bass as bass
import concourse.mybir as mybir
import concourse.tile as tile
import concourse.bass_utils as bu
from concourse.masks import make_identity

bu.upload_artifacts = lambda tmpdir: 'local://' + tmpdir

bf = ml_dtypes.bfloat16

N, E, D = 50000, 500000, 128
NREL, NCLS = 64, 16
NC = 8
NPC = N // NC            # 6250
NB = 49                  # 128-node blocks per core
NODES_PAD = NB * 128     # 6272
NPAD_G = 50176           # 392 * 128
MAXTPB = 14
BLK_CAP = MAXTPB * 128
RUNL = 8                 # rows streamed per index in a run-call
CAP = 3                  # max table-row copies per node per core
GBLK = 7                 # blocks per run-table group
GRP = NB // GBLK         # 7 run-table groups

dt = mybir.dt
F32 = dt.float32
I32 = dt.int32
BF16 = dt.bfloat16

TRACE = False
LAST_EXEC_NS = {}


def _split_multiwaits(nc, max_waits=1):
    for b in nc.m.functions[0].blocks:
        old = list(b.instructions)
        new = []
        changed = False
        for inst in old:
            si = inst.sync_info
            if si is not None and len(si.on_wait) > max_waits:
                waits = list(si.on_wait)
                chunks = [waits[i:i + max_waits]
                          for i in range(0, len(waits), max_waits)]
                for k, ch in enumerate(chunks[:-1]):
                    new.append(mybir.InstNoOp(
                        name=f"{inst.name}_sw{k}", engine=inst.engine,
                        bass_nofuse=True,
                        sync_info=mybir.SyncInfo(on_wait=ch, on_update=[])))
                inst.sync_info = mybir.SyncInfo(
                    on_wait=chunks[-1], on_update=list(si.on_update))
                changed = True
            new.append(inst)
        if changed:
            b.instructions = new


def _make_nc():
    return bass.Bass("TRN2", target_bir_lowering=False, debug=False,
                     num_devices=NC)


def _build_A(CALLS, CB, TS, TILES, GRPA, SROWS):
    """CALLS[b]: list of (c0, L, ixcol); CB[b]: cols; TS[b]: col start.
    GRPA[g]: padded rows of run-table g."""
    nc = _make_nc()
    NIX = max(ix for calls in CALLS for (_, _, ix) in calls) + 1
    nfTg = [nc.dram_tensor(f"nfTg{g}", (D, GRPA[g]), BF16,
                           kind="ExternalInput") for g in range(GRP)]
    nfTs = nc.dram_tensor("nfTs", (D, SROWS), BF16, kind="ExternalInput")
    nfTl = nc.dram_tensor("nfTl", (D, NODES_PAD), BF16, kind="ExternalInput")
    W1mT = nc.dram_tensor("W1mT", (D, D), BF16, kind="ExternalInput")
    W2mT = nc.dram_tensor("W2mT", (D, D), BF16, kind="ExternalInput")
    Rrel = nc.dram_tensor("Rrel", (NREL, D), BF16, kind="ExternalInput")
    wihT = nc.dram_tensor("wihT", (D, 3 * D), BF16, kind="ExternalInput")
    whhT = nc.dram_tensor("whhT", (D, 3 * D), BF16, kind="ExternalInput")
    gbr = nc.dram_tensor("gbr", (D, 1), F32, kind="ExternalInput")
    gbz = nc.dram_tensor("gbz", (D, 1), F32, kind="ExternalInput")
    gbin = nc.dram_tensor("gbin", (D, 1), F32, kind="ExternalInput")
    gbhn = nc.dram_tensor("gbhn", (D, 1), F32, kind="ExternalInput")
    W1cT = nc.dram_tensor("W1cT", (D, D), BF16, kind="ExternalInput")
    W2cT = nc.dram_tensor("W2cT", (D, D), BF16, kind="ExternalInput")
    srcix = nc.dram_tensor("srcix", (D, NIX), I32, kind="ExternalInput")
    trl = nc.dram_tensor("trl", (D, TILES), F32, kind="ExternalInput")
    trelB = nc.dram_tensor("trelB", (D, TILES * D), BF16, kind="ExternalInput")
    labB = nc.dram_tensor("labB", (NREL, TILES * D), BF16, kind="ExternalInput")
    U1s = nc.dram_tensor("U1s", (NB, D, D), BF16, kind="ExternalOutput")
    U2s = nc.dram_tensor("U2s", (NB, D, D), BF16, kind="ExternalOutput")
    rtab = [nc.dram_tensor(f"rt{g}", (GRPA[g], D), BF16, kind="Internal")
            for g in range(GRP)]
    stab = nc.dram_tensor("stab", (SROWS, D), BF16, kind="Internal")

    with tile.TileContext(nc) as tc:
        with tc.tile_pool(name="const", bufs=1) as cp:
            w1m = cp.tile([D, D], BF16); nc.sync.dma_start(w1m[:], W1mT[:])
            w2m = cp.tile([D, D], BF16); nc.sync.dma_start(w2m[:], W2mT[:])
            wih = cp.tile([D, 3 * D], BF16); nc.sync.dma_start(wih[:], wihT[:])
            whh = cp.tile([D, 3 * D], BF16); nc.sync.dma_start(whh[:], whhT[:])
            gbrt = cp.tile([D, 1], F32); nc.sync.dma_start(gbrt[:], gbr[:])
            gbzt = cp.tile([D, 1], F32); nc.sync.dma_start(gbzt[:], gbz[:])
            gin = cp.tile([D, 1], F32); nc.sync.dma_start(gin[:], gbin[:])
            ghn = cp.tile([D, 1], F32); nc.sync.dma_start(ghn[:], gbhn[:])
            w1c = cp.tile([D, D], BF16); nc.sync.dma_start(w1c[:], W1cT[:])
            w2c = cp.tile([D, D], BF16); nc.sync.dma_start(w2c[:], W2cT[:])
            rrl = cp.tile([NREL, D], BF16); nc.sync.dma_start(rrl[:], Rrel[:])
            six = cp.tile([D, NIX], I32); nc.sync.dma_start(six[:], srcix[:])
            trlt = cp.tile([D, TILES], F32); nc.sync.dma_start(trlt[:], trl[:])
            nfl = cp.tile([D, NODES_PAD], BF16)
            nc.sync.dma_start(nfl[:], nfTl[:])
            iotac = cp.tile([D, 1], F32)
            nc.gpsimd.iota(iotac[:], pattern=[[0, 1]], base=0,
                           channel_multiplier=1,
                           allow_small_or_imprecise_dtypes=True)
            iotR = cp.tile([D, D], F32)   # row-iota: iotR[p, c] = c
            nc.gpsimd.iota(iotR[:], pattern=[[1, D]], base=0,
                           channel_multiplier=0,
                           allow_small_or_imprecise_dtypes=True)
            identE = cp.tile([D, D], BF16)
            make_identity(nc, identE[:])
            w2loc = cp.tile([D, NB * D], BF16)   # local nfW2 blocks [n, d]

            # ---- phase A1: per-core nfW1 gather tables ----
            # build order rt0, stab, rt1..rt6: block 0's run gather waits
            # only on rt0 (~25us), its singles on stab, etc.
            with tc.tile_pool(name="pa", bufs=3) as pa, \
                 tc.tile_pool(name="ps_a", bufs=4, space="PSUM") as ps_a:
                border = [(rtab[0], nfTg[0], GRPA[0]), (stab, nfTs, SROWS)]
                border += [(rtab[g], nfTg[g], GRPA[g]) for g in range(1, GRP)]
                for (dstT, srcT, rows) in border:
                    for i in range((rows + 2047) // 2048):
                        c0 = i * 2048
                        cw = min(2048, rows - c0)
                        ch = pa.tile([D, 2048], BF16)
                        nc.sync.dma_start(ch[:, 0:cw], srcT[:, c0:c0 + cw])
                        st = pa.tile([D, 2048], BF16)
                        for h in range(cw // 512):
                            p1 = ps_a.tile([D, 512], F32, space="PSUM")
                            for j in range(4):
                                nc.tensor.matmul(
                                    out=p1[:, j * D:(j + 1) * D],
                                    lhsT=ch[:, (h * 4 + j) * D:
                                            (h * 4 + j + 1) * D],
                                    rhs=w1m[:], start=True, stop=True)
                            if h % 2 == 0:
                                nc.scalar.copy(
                                    st[:, h * 512:(h + 1) * 512], p1[:])
                            else:
                                nc.vector.tensor_copy(
                                    st[:, h * 512:(h + 1) * 512], p1[:])
                        nc.sync.dma_start(
                            dstT[c0:c0 + cw, :].rearrange(
                                "(b p) d -> p b d", p=D), st[:, 0:cw])
                # ---- phase A2: local nfW2 blocks into SBUF ----
                for i in range(NB // 4 + 1):
                    nblk = min(4, NB - i * 4)
                    if nblk <= 0:
                        break
                    p1 = ps_a.tile([D, 512], F32, space="PSUM")
                    for j in range(nblk):
                        b = i * 4 + j
                        nc.tensor.matmul(
                            out=p1[:, j * D:(j + 1) * D],
                            lhsT=nfl[:, b * D:(b + 1) * D],
                            rhs=w2m[:], start=True, stop=True)
                    nc.vector.tensor_copy(
                        w2loc[:, i * 512:i * 512 + nblk * D],
                        p1[:, 0:nblk * D])

            # ---- phase B: edges + GRU + U tables ----
            with tc.tile_pool(name="pg", bufs=6) as pg, \
                 tc.tile_pool(name="prow", bufs=6) as prow, \
                 tc.tile_pool(name="poh", bufs=4) as poh, \
                 tc.tile_pool(name="pmsg", bufs=3) as pmsg, \
                 tc.tile_pool(name="pgr", bufs=4) as pgr, \
                 tc.tile_pool(name="pub", bufs=3) as pub, \
                 tc.tile_pool(name="ps_pre", bufs=3, space="PSUM") as ps_pre, \
                 tc.tile_pool(name="ps_agg", bufs=3, space="PSUM") as ps_agg, \
                 tc.tile_pool(name="ps_g4", bufs=1, space="PSUM") as ps_g4, \
                 tc.tile_pool(name="ps_u", bufs=1, space="PSUM") as ps_u:
                for b in range(NB):
                    tpb = CB[b]
                    ts = TS[b]
                    nsup = (tpb + 3) // 4
                    g1 = pg.tile([D, tpb * D], BF16)
                    for (c0, L, ixcol) in CALLS[b]:
                        srctab = rtab[b // GBLK] if L > 1 else stab
                        nc.gpsimd.indirect_dma_start(
                            out=g1[:, c0 * D:(c0 + L) * D], out_offset=None,
                            in_=srctab[:],
                            in_offset=bass.IndirectOffsetOnAxis(
                                ap=six[:, ixcol:ixcol + 1], axis=0))
                    trB = prow.tile([D, tpb * D], BF16)
                    nc.sync.dma_start(
                        trB[:], trelB[:, ts * D:(ts + tpb) * D])
                    lbB = prow.tile([NREL, tpb * D], BF16)
                    nc.sync.dma_start(
                        lbB[:], labB[:, ts * D:(ts + tpb) * D])
                    agg = ps_agg.tile([D, D], F32, space="PSUM")
                    for s in range(nsup):
                        t0 = s * 4
                        nt = min(4, tpb - t0)
                        w = nt * D
                        ohs = []
                        for (bsrc, npart) in ((trB, D), (lbB, NREL)):
                            oht_ = poh.tile([D, 512], BF16)
                            nc.vector.tensor_scalar(
                                out=oht_[0:npart, 0:w],
                                in0=bsrc[0:npart, t0 * D:t0 * D + w],
                                scalar1=iotac[0:npart, 0:1], scalar2=None,
                                op0=mybir.AluOpType.is_equal)
                            ohs.append(oht_)
                        ohT, ohrT = ohs
                        oh4 = poh.tile([D, 512], BF16)
                        nc.vector.tensor_tensor(
                            out=oh4[:, 0:w].rearrange("p (t d) -> p t d", t=nt),
                            in0=trlt[:, ts + t0:ts + t0 + nt]
                                .unsqueeze(2).broadcast_to((D, nt, D)),
                            in1=iotR[:].unsqueeze(1).broadcast_to((D, nt, D)),
                            op=mybir.AluOpType.is_equal)
                        pre = ps_pre.tile([D, 512], F32, space="PSUM")
                        for j in range(nt):
                            nc.tensor.matmul(
                                out=pre[:, j * D:(j + 1) * D],
                                lhsT=ohT[:, j * D:(j + 1) * D],
                                rhs=w2loc[:, b * D:(b + 1) * D],
                                start=True, stop=False)
                            nc.tensor.matmul(
                                out=pre[:, j * D:(j + 1) * D],
                                lhsT=ohrT[0:NREL, j * D:(j + 1) * D],
                                rhs=rrl[:], start=False, stop=True)
                        s2 = pmsg.tile([D, 512], BF16)
                        nc.vector.tensor_tensor(
                            out=s2[:, 0:w], in0=pre[:, 0:w],
                            in1=g1[:, t0 * D:t0 * D + w],
                            op=mybir.AluOpType.add)
                        msg4 = pmsg.tile([D, 512], BF16)
                        nc.scalar.activation(
                            msg4[:, 0:w], s2[:, 0:w],
                            mybir.ActivationFunctionType.Gelu)
                        for j in range(nt):
                            t = t0 + j
                            nc.tensor.matmul(
                                out=agg[:], lhsT=msg4[:, j * D:(j + 1) * D],
                                rhs=oh4[:, j * D:(j + 1) * D],
                                start=(t == 0), stop=(t == tpb - 1))
                    # ---- GRU (per block, overlaps the gather stream) ----
                    nfb = nfl[:, b * D:(b + 1) * D]
                    aggs = pgr.tile([D, D], BF16)
                    nc.scalar.copy(aggs[:], agg[:])
                    g4 = ps_g4.tile([D, 4 * D], F32, space="PSUM")
                    nc.tensor.matmul(out=g4[:, 0:D], lhsT=wih[:, 0:D],
                                     rhs=aggs[:], start=True, stop=False)
                    nc.tensor.matmul(out=g4[:, 0:D], lhsT=whh[:, 0:D],
                                     rhs=nfb, start=False, stop=True)
                    nc.tensor.matmul(out=g4[:, D:2 * D], lhsT=wih[:, D:2 * D],
                                     rhs=aggs[:], start=True, stop=False)
                    nc.tensor.matmul(out=g4[:, D:2 * D], lhsT=whh[:, D:2 * D],
                                     rhs=nfb, start=False, stop=True)
                    nc.tensor.matmul(out=g4[:, 2 * D:3 * D],
                                     lhsT=wih[:, 2 * D:3 * D],
                                     rhs=aggs[:], start=True, stop=True)
                    nc.tensor.matmul(out=g4[:, 3 * D:4 * D],
                                     lhsT=whh[:, 2 * D:3 * D],
                                     rhs=nfb, start=True, stop=True)
                    rz = pgr.tile([D, 2 * D], BF16)
                    nc.scalar.activation(rz[:, 0:D], g4[:, 0:D],
                                         mybir.ActivationFunctionType.Sigmoid,
                                         bias=gbrt[:, 0:1])
                    nc.scalar.activation(rz[:, D:2 * D], g4[:, D:2 * D],
                                         mybir.ActivationFunctionType.Sigmoid,
                                         bias=gbzt[:, 0:1])
                    r = rz[:, 0:D]
                    z = rz[:, D:2 * D]
                    hn = pgr.tile([D, D], BF16)
                    nc.vector.tensor_scalar(
                        out=hn[:], in0=g4[:, 3 * D:4 * D],
                        scalar1=ghn[:, 0:1], scalar2=None,
                        op0=mybir.AluOpType.add)
                    t1 = pgr.tile([D, D], BF16)
                    nc.vector.tensor_tensor(out=t1[:], in0=r, in1=hn[:],
                                            op=mybir.AluOpType.mult)
                    t2 = pgr.tile([D, D], BF16)
                    nc.vector.tensor_tensor(out=t2[:], in0=t1[:],
                                            in1=g4[:, 2 * D:3 * D],
                                            op=mybir.AluOpType.add)
                    n_ = pgr.tile([D, D], BF16)
                    nc.scalar.activation(n_[:], t2[:],
                                         mybir.ActivationFunctionType.Tanh,
                                         bias=gin[:, 0:1])
                    d1 = pgr.tile([D, D], BF16)
                    nc.vector.tensor_tensor(out=d1[:], in0=nfb, in1=n_[:],
                                            op=mybir.AluOpType.subtract)
                    d2 = pgr.tile([D, D], BF16)
                    nc.vector.tensor_tensor(out=d2[:], in0=z[:], in1=d1[:],
                                            op=mybir.AluOpType.mult)
                    nfu = pgr.tile([D, D], BF16)
                    nc.vector.tensor_tensor(out=nfu[:], in0=n_[:], in1=d2[:],
                                            op=mybir.AluOpType.add)
                    pu = ps_u.tile([D, 2 * D], F32, space="PSUM")
                    nc.tensor.matmul(out=pu[:, 0:D], lhsT=nfu[:], rhs=w1c[:],
                                     start=True, stop=True)
                    nc.tensor.matmul(out=pu[:, D:2 * D], lhsT=nfu[:],
                                     rhs=w2c[:], start=True, stop=True)
                    u1 = pub.tile([D, D], BF16)
                    nc.scalar.copy(u1[:], pu[:, 0:D])
                    nc.sync.dma_start(U1s[b], u1[:])
                    u2 = pub.tile([D, D], BF16)
                    nc.scalar.copy(u2[:], pu[:, D:2 * D])
                    nc.sync.dma_start(U2s[b], u2[:])

    _split_multiwaits(nc)
    return nc


def _build_B(CALLS, CB, TS, TILES, TROWS):
    nc = _make_nc()
    NIX = max(ix for calls in CALLS for (_, _, ix) in calls) + 1
    U1 = nc.dram_tensor("U1", (TROWS, D), BF16, kind="ExternalInput")
    U2l = nc.dram_tensor("U2l", (NB, D, D), BF16, kind="ExternalInput")
    efT = nc.dram_tensor("efT", (D, TILES * D), BF16, kind="ExternalInput")
    srcix = nc.dram_tensor("srcix", (D, NIX), I32, kind="ExternalInput")
    trelB = nc.dram_tensor("trelB", (D, TILES * D), BF16, kind="ExternalInput")
    W3cT = nc.dram_tensor("W3cT", (D, D), BF16, kind="ExternalInput")
    clsW2 = nc.dram_tensor("clsW2", (D, NCLS), BF16, kind="ExternalInput")
    b1 = nc.dram_tensor("b1", (D, 1), F32, kind="ExternalInput")
    b2 = nc.dram_tensor("b2", (NCLS, 1), F32, kind="ExternalInput")
    outT = nc.dram_tensor("outT", (NCLS, TILES * D), F32,
                          kind="ExternalOutput")

    with tile.TileContext(nc) as tc:
        with tc.tile_pool(name="const", bufs=1) as cp, \
             tc.tile_pool(name="pg", bufs=6) as pg, \
             tc.tile_pool(name="prow", bufs=6) as prow, \
             tc.tile_pool(name="pef", bufs=5) as pef, \
             tc.tile_pool(name="poh", bufs=4) as poh, \
             tc.tile_pool(name="phc", bufs=3) as phc, \
             tc.tile_pool(name="pu2", bufs=4) as pu2, \
             tc.tile_pool(name="pout", bufs=3) as pout, \
             tc.tile_pool(name="ps_x", bufs=3, space="PSUM") as ps_x, \
             tc.tile_pool(name="ps_p", bufs=3, space="PSUM") as ps_p:
            w3 = cp.tile([D, D], BF16); nc.sync.dma_start(w3[:], W3cT[:])
            w2 = cp.tile([D, NCLS], BF16); nc.sync.dma_start(w2[:], clsW2[:])
            b1t = cp.tile([D, 1], F32); nc.sync.dma_start(b1t[:], b1[:])
            b2t = cp.tile([NCLS, 1], F32); nc.sync.dma_start(b2t[:], b2[:])
            six = cp.tile([D, NIX], I32); nc.sync.dma_start(six[:], srcix[:])
            iotac = cp.tile([D, 1], F32)
            nc.gpsimd.iota(iotac[:], pattern=[[0, 1]], base=0,
                           channel_multiplier=1,
                           allow_small_or_imprecise_dtypes=True)
            identE = cp.tile([D, D], BF16)
            make_identity(nc, identE[:])

            for b in range(NB):
                tpb = CB[b]
                ts = TS[b]
                nsup = (tpb + 3) // 4
                g1 = pg.tile([D, tpb * D], BF16)
                for (c0, L, ixcol) in CALLS[b]:
                    nc.gpsimd.indirect_dma_start(
                        out=g1[:, c0 * D:(c0 + L) * D], out_offset=None,
                        in_=U1[:],
                        in_offset=bass.IndirectOffsetOnAxis(
                            ap=six[:, ixcol:ixcol + 1], axis=0))
                u2b = pu2.tile([D, D], BF16)
                nc.sync.dma_start(u2b[:], U2l[b])
                ef = pef.tile([D, tpb * D], BF16)
                nc.sync.dma_start(ef[:], efT[:, ts * D:(ts + tpb) * D])
                trB = prow.tile([D, tpb * D], BF16)
                nc.sync.dma_start(trB[:], trelB[:, ts * D:(ts + tpb) * D])
                ob = pout.tile([NCLS, tpb * D], F32)
                for s in range(nsup):
                    t0 = s * 4
                    nt = min(4, tpb - t0)
                    w = nt * D
                    ohT = poh.tile([D, 512], BF16)
                    nc.vector.tensor_scalar(
                        out=ohT[:, 0:w],
                        in0=trB[:, t0 * D:t0 * D + w],
                        scalar1=iotac[:, 0:1], scalar2=None,
                        op0=mybir.AluOpType.is_equal)
                    xt = ps_x.tile([D, 512], F32, space="PSUM")
                    nc.tensor.matmul(
                        out=xt[:, 0:w], lhsT=w3[:],
                        rhs=ef[:, t0 * D:t0 * D + w], start=True, stop=False)
                    nc.tensor.matmul(
                        out=xt[:, 0:w], lhsT=u2b[:],
                        rhs=ohT[:, 0:w], start=False, stop=False)
                    for j in range(nt):
                        t = t0 + j
                        nc.tensor.matmul(
                            out=xt[:, j * D:(j + 1) * D],
                            lhsT=g1[:, t * D:(t + 1) * D],
                            rhs=identE[:], start=False,
                            stop=(j == nt - 1), skip_group_check=True)
                    hc = phc.tile([D, 512], BF16)
                    nc.scalar.activation(
                        hc[:, 0:w], xt[:, 0:w],
                        mybir.ActivationFunctionType.Gelu, bias=b1t[:, 0:1])
                    po = ps_p.tile([NCLS, 512], F32, space="PSUM")
                    nc.tensor.matmul(out=po[:, 0:w], lhsT=w2[:],
                                     rhs=hc[:, 0:w], start=True, stop=True)
                    nc.scalar.add(ob[:, t0 * D:t0 * D + w], po[:, 0:w],
                                  b2t[:, 0:1])
                nc.sync.dma_start(outT[:, ts * D:(ts + tpb) * D], ob[:])

    _split_multiwaits(nc)
    return nc


_CACHE = {}


def _get(name, builder, *args):
    if name not in _CACHE:
        _CACHE[name] = builder(*args)
    return _CACHE[name]


def _run(nc, in_maps, tag):
    kw = {}
    if TRACE:
        import tempfile
        kw = dict(trace=True, tmpdir=tempfile.mkdtemp(prefix=f"gcn2_{tag}_"))
    res = bu.run_bass_kernel_spmd(nc, in_maps, core_ids=list(range(NC)), **kw)
    if TRACE:
        LAST_EXEC_NS[tag] = res.exec_time_ns
        LAST_EXEC_NS[tag + "_dir"] = kw["tmpdir"]
    return res.results


def kernel(node_features, edge_features, edge_index, labels_for_rel_emb,
           rel_emb, msg_W, msg_b, gru_w_ih, gru_w_hh, gru_b_ih, gru_b_hh,
           cls_W1, cls_b1, cls_W2, cls_b2):
    nf = np.asarray(node_features, np.float32)
    ef = np.asarray(edge_features, np.float32)
    src = np.asarray(edge_index[0], np.int64).astype(np.int32)
    tgt = np.asarray(edge_index[1], np.int64).astype(np.int32)
    lab = np.asarray(labels_for_rel_emb, np.int64).astype(np.int32)
    msg_W = np.asarray(msg_W, np.float32)
    cls_W1 = np.asarray(cls_W1, np.float32)

    # ---- host: relabel nodes so per-block edge counts pack tightly ----
    deg = np.bincount(tgt, minlength=N)
    byd = np.argsort(-deg, kind="stable")
    corder = np.concatenate([np.r_[0:NC, NC - 1:-1:-1]
                             for _ in range((N + 2 * NC - 1) // (2 * NC))])
    node_core = np.empty(N, np.int32)
    node_core[byd] = corder[:N]
    perm = np.empty(N, np.int64)  # old id -> new (padded-global) id
    for c in range(NC):
        nodes_c = byd[node_core[byd] == c]
        degs_c = deg[nodes_c]
        cap_e = np.full(NB, float(max(1280, int(np.ceil(degs_c.sum() / NB)))))
        cnt_n = np.zeros(NB, np.int64)
        sum_e = np.zeros(NB, np.float64)
        blk_of = np.empty(len(nodes_c), np.int64)
        for i, dg in enumerate(degs_c):
            cand = np.where((cnt_n < 128) & (sum_e + dg <= cap_e))[0]
            if len(cand) == 0:
                cand = np.where(cnt_n < 128)[0]
                j = cand[np.argmin(sum_e[cand])]
            else:
                j = cand[np.argmax(sum_e[cand])]
            blk_of[i] = j
            cnt_n[j] += 1
            sum_e[j] += dg
        ordb = np.argsort(blk_of, kind="stable")
        pcount = np.zeros(NB, np.int64)
        newloc = np.empty(len(nodes_c), np.int64)
        for i in ordb:
            j = blk_of[i]
            newloc[i] = j * 128 + pcount[j]
            pcount[j] += 1
        perm[nodes_c] = c * NODES_PAD + newloc
    src = perm[src].astype(np.int32)
    tgt = perm[tgt].astype(np.int32)
    nf_pad = np.zeros((NC * NODES_PAD, D), np.float32)
    nf_pad[perm] = nf
    nf = nf_pad

    order = np.argsort(tgt, kind="stable")
    tgt_s = tgt[order]; src_s = src[order]; lab_s = lab[order]
    ef_s = ef[order]
    core_s = tgt_s // NODES_PAD
    blk_s = (tgt_s - core_s * NODES_PAD) // 128
    key = core_s * NB + blk_s
    gstart = np.searchsorted(key, np.arange(NC * NB + 1))
    counts = np.diff(gstart).reshape(NC, NB)

    # src occurrence rank within each core's tgt-sorted edge stream:
    # occ < CAP -> edge owns a fresh table-row copy of its src
    occ = np.zeros(E, np.int64)
    for c in range(NC):
        lo, hi = gstart[c * NB], gstart[(c + 1) * NB] if c + 1 < NC else gstart[NC * NB]
        s_c = src_s[lo:hi]
        so = np.argsort(s_c, kind="stable")
        ss = s_c[so]
        grp = np.concatenate([[0], np.cumsum(ss[1:] != ss[:-1])])
        first = np.concatenate([[0], np.flatnonzero(ss[1:] != ss[:-1]) + 1])
        occ_sorted = np.arange(len(ss)) - first[grp]
        o = np.empty(len(ss), np.int64)
        o[so] = occ_sorted
        occ[lo:hi] = o

    # ---- per-core run/single packing ----
    # per block: pick run length LB[b] (8/4/2) so even thin blocks ride
    # run-calls; NRUN/NSING = max over cores for the shared SPMD structure.
    COB = np.zeros((NC, NB), np.int64)
    for c in range(NC):
        for b in range(NB):
            lo, hi = gstart[c * NB + b], gstart[c * NB + b + 1]
            COB[c, b] = np.count_nonzero(occ[lo:hi] < CAP)
    LB = []
    NRUN = []
    for b in range(NB):
        mx = int(COB[:, b].max())
        for L in (8, 4, 2):
            if mx >= 128 * L:
                break
        LB.append(L)
        NRUN.append(max(1, mx // (128 * L)) if mx >= 128 * L else 0)
    NSING = [0] * NB
    for b in range(NB):
        need = 0
        rs = 128 * LB[b]
        for c in range(NC):
            nedge = int(gstart[c * NB + b + 1] - gstart[c * NB + b])
            placed_full = (min(int(COB[c, b]), NRUN[b] * rs) // LB[b]) * LB[b]
            rest = nedge - placed_full
            need = max(need, (rest + 127) // 128)
        NSING[b] = need

    CB = [NRUN[b] * LB[b] + NSING[b] for b in range(NB)]
    assert max(CB) <= MAXTPB, f"block cols overflow: {max(CB)}"
    TS = np.concatenate([[0], np.cumsum(CB)]).astype(int)
    TILES = int(TS[NB])
    TS = TS[:NB]
    SLOTS = TILES * 128

    # call list (shared across cores): per block: run-calls then single-calls
    CALLS = []
    ixcol = 0
    for b in range(NB):
        calls_b = []
        for r in range(NRUN[b]):
            calls_b.append((r * LB[b], LB[b], ixcol)); ixcol += 1
        for s_ in range(NSING[b]):
            calls_b.append((NRUN[b] * LB[b] + s_, 1, ixcol)); ixcol += 1
        CALLS.append(calls_b)
    NIX = ixcol

    # ---- per-core tables, indices, slot metadata ----
    # run rows live in per-group tables (group g = blocks 7g..7g+6);
    # single-call indices point into a separate singles table.
    GROUPLISTS = [[[] for _ in range(GRP)] for _ in range(NC)]
    SINGLISTS = [[] for _ in range(NC)]
    SPOS = [{} for _ in range(NC)]    # node id -> singles-table row
    IXCS = np.zeros((NC, D, NIX), np.int32)
    TRELP = np.full((NC, SLOTS), -1.0, np.float32)
    LABF = np.full((NC, SLOTS), -1.0, np.float32)
    EPOS = np.full(E, -1, np.int64)   # edge (tgt-sorted idx) -> core*SLOTS+slot
    for c in range(NC):
        for b in range(NB):
            grp = b // GBLK
            Lb = LB[b]
            rs = 128 * Lb
            rowlist = GROUPLISTS[c][grp]
            lo, hi = gstart[c * NB + b], gstart[c * NB + b + 1]
            eb = np.arange(lo, hi)
            isco = occ[lo:hi] < CAP
            co_e = eb[isco]
            oth_e = eb[~isco]
            placed = min(len(co_e), NRUN[b] * rs)
            run_e = co_e[:placed]
            rest_e = np.concatenate([co_e[placed:], oth_e])
            ci = 0
            # run-calls
            for r in range(NRUN[b]):
                seg = run_e[r * rs:(r + 1) * rs]
                base = len(rowlist)
                for p in range(128):
                    run = seg[p * Lb:(p + 1) * Lb]
                    if len(run) == Lb:
                        IXCS[c, p, CALLS[b][ci][2]] = base + p * Lb
                        for j, e_ in enumerate(run):
                            col = TS[b] + r * Lb + j
                            slot = c * SLOTS + col * 128 + p
                            TRELP[c, col * 128 + p] = tgt_s[e_] - (
                                c * NODES_PAD + b * 128)
                            LABF[c, col * 128 + p] = lab_s[e_]
                            EPOS[e_] = slot
                            rowlist.append(int(src_s[e_]))
                    else:
                        IXCS[c, p, CALLS[b][ci][2]] = 0   # dummy run
                        # partial run edges go to rest
                        rest_e = np.concatenate([rest_e, run])
                        # pad rowlist so later runs keep alignment
                        rowlist.extend([0] * Lb)
                ci += 1
            # single-calls (indices into the singles table)
            singlist = SINGLISTS[c]
            spos = SPOS[c]
            for s_ in range(NSING[b]):
                seg = rest_e[s_ * 128:(s_ + 1) * 128]
                col = TS[b] + NRUN[b] * LB[b] + s_
                callix = CALLS[b][ci][2]
                for p in range(128):
                    if p < len(seg):
                        e_ = seg[p]
                        TRELP[c, col * 128 + p] = tgt_s[e_] - (
                            c * NODES_PAD + b * 128)
                        LABF[c, col * 128 + p] = lab_s[e_]
                        EPOS[e_] = c * SLOTS + col * 128 + p
                        s_n = int(src_s[e_])
                        if s_n not in spos:
                            spos[s_n] = len(singlist)
                            singlist.append(s_n)
                        IXCS[c, p, callix] = spos[s_n]
                    # else: stays masked, index 0
                ci += 1
    assert EPOS.min() >= 0, "some edges were not assigned slots"

    # group-table sizes are core-independent by construction
    GRPA = [max(512, ((len(GROUPLISTS[0][g]) + 511) // 512) * 512)
            for g in range(GRP)]
    for c in range(NC):
        for g in range(GRP):
            assert len(GROUPLISTS[c][g]) == len(GROUPLISTS[0][g])
    SROWS = max(1, max(len(sl) for sl in SINGLISTS))
    SROWS = ((SROWS + 511) // 512) * 512

    nfT16 = np.ascontiguousarray(nf.T).astype(bf)   # [D, NC*NODES_PAD]
    W1mT = np.ascontiguousarray(msg_W[:, 0:D].T).astype(bf)
    W2mT = np.ascontiguousarray(msg_W[:, D:2 * D].T).astype(bf)
    Rrel = (np.asarray(rel_emb, np.float32) @ msg_W[:, 2 * D:3 * D].T
            + np.asarray(msg_b, np.float32)).astype(bf)
    wihT = np.ascontiguousarray(np.asarray(gru_w_ih, np.float32).T).astype(bf)
    whhT = np.ascontiguousarray(np.asarray(gru_w_hh, np.float32).T).astype(bf)
    bih = np.asarray(gru_b_ih, np.float32)
    bhh = np.asarray(gru_b_hh, np.float32)
    gbr = (bih[0:D] + bhh[0:D]).reshape(D, 1).astype(np.float32)
    gbz = (bih[D:2 * D] + bhh[D:2 * D]).reshape(D, 1).astype(np.float32)
    gbin = bih[2 * D:3 * D].reshape(D, 1).astype(np.float32)
    gbhn = bhh[2 * D:3 * D].reshape(D, 1).astype(np.float32)
    W1cT = np.ascontiguousarray(cls_W1[:, 0:D].T).astype(bf)
    W2cT = np.ascontiguousarray(cls_W1[:, D:2 * D].T).astype(bf)
    W3cT = np.ascontiguousarray(cls_W1[:, 2 * D:3 * D].T).astype(bf)

    def col_layout(a):
        return np.ascontiguousarray(a.reshape(TILES, 128).T)

    # padded per-table row-id arrays (node ids; 0-padded)
    GROWARR = []   # [c][g] -> int64 [GRPA[g]]
    SROWARR = []   # [c] -> int64 [SROWS]
    for c in range(NC):
        gl = []
        for g in range(GRP):
            ra = np.zeros(GRPA[g], np.int64)
            rl = GROUPLISTS[c][g]
            ra[:len(rl)] = np.asarray(rl, np.int64)
            gl.append(ra)
        GROWARR.append(gl)
        sa = np.zeros(SROWS, np.int64)
        sl_l = SINGLISTS[c]
        sa[:len(sl_l)] = np.asarray(sl_l, np.int64)
        SROWARR.append(sa)

    in_maps_A = []
    for c in range(NC):
        nfTl = nfT16[:, c * NODES_PAD:(c + 1) * NODES_PAD]
        m = {
            "nfTl": np.ascontiguousarray(nfTl), "W1mT": W1mT,
            "W2mT": W2mT, "Rrel": Rrel, "wihT": wihT, "whhT": whhT,
            "gbr": gbr, "gbz": gbz, "gbin": gbin, "gbhn": gbhn,
            "W1cT": W1cT, "W2cT": W2cT,
            "srcix": np.ascontiguousarray(IXCS[c]),
            "trl": np.ascontiguousarray(col_layout(TRELP[c])),
            "trelB": np.ascontiguousarray(np.broadcast_to(
                TRELP[c].astype(bf)[None, :], (D, SLOTS))),
            "labB": np.ascontiguousarray(np.broadcast_to(
                LABF[c].astype(bf)[None, :], (NREL, SLOTS))),
        }
        for g in range(GRP):
            m[f"nfTg{g}"] = np.ascontiguousarray(nfT16[:, GROWARR[c][g]])
        m["nfTs"] = np.ascontiguousarray(nfT16[:, SROWARR[c]])
        in_maps_A.append(m)

    ncA = _get("A", _build_A, CALLS, CB, TS, TILES, GRPA, SROWS)
    resA = _run(ncA, in_maps_A, "A")

    U1 = np.concatenate(
        [np.asarray(resA[c]["U1s"]).reshape(NODES_PAD, D)
         for c in range(NC)], axis=0)    # [NC*NODES_PAD, D], padded-global ids

    clsW2 = np.ascontiguousarray(np.asarray(cls_W2, np.float32).T).astype(bf)
    b1v = np.asarray(cls_b1, np.float32).reshape(D, 1)
    b2v = np.asarray(cls_b2, np.float32).reshape(NCLS, 1)

    EFP = np.zeros((NC * SLOTS, D), np.float32)
    EFP[EPOS] = ef_s

    # B uses one concatenated table [rt0|rt1|..|rt6|singles]; rebase the
    # per-call indices by each call's table offset.
    GRPOFF = np.concatenate([[0], np.cumsum(GRPA)]).astype(np.int64)
    TROWSB = int(GRPOFF[GRP] + SROWS)
    IXBASE = np.zeros(NIX, np.int32)
    for b in range(NB):
        for (c0, L, ixcol) in CALLS[b]:
            IXBASE[ixcol] = (GRPOFF[b // GBLK] if L > 1
                             else GRPOFF[GRP])
    in_maps_B = []
    for c in range(NC):
        sl = slice(c * SLOTS, (c + 1) * SLOTS)
        u2 = np.asarray(resA[c]["U2s"])
        rowcat = np.concatenate(GROWARR[c] + [SROWARR[c]])
        in_maps_B.append({
            "U1": np.ascontiguousarray(U1[rowcat]).astype(bf), "U2l": u2,
            "efT": np.ascontiguousarray(EFP[sl].T).astype(bf),
            "srcix": np.ascontiguousarray(IXCS[c] + IXBASE[None, :]),
            "trelB": in_maps_A[c]["trelB"],
            "W3cT": W3cT, "clsW2": clsW2, "b1": b1v, "b2": b2v,
        })

    ncB = _get("B", _build_B, CALLS, CB, TS, TILES, TROWSB)
    resB = _run(ncB, in_maps_B, "B")

    outS = np.concatenate(
        [np.asarray(resB[c]["outT"]).T for c in range(NC)], axis=0)
    out_sorted = outS[EPOS]           # [E, NCLS] in tgt-sorted edge order
    out = np.empty((E, NCLS), np.float32)
    out[order] = out_sorted
    return np.ascontiguousarray(out.astype(np.float32))
